# revision 1
# baseline (speedup 1.0000x reference)
"""Trainium2 Bass kernel for y = 2*(einsum('bct,oc->bot', pre, W_pre) + b_pre).

Shapes (hardcoded): pre [16, 512, 4096] f32, W_pre [512, 512] f32, b_pre [512] f32.
Sharding: data-parallel over B across 8 cores (2 batches per core).

Per core: out[b, o, t] = 2*(sum_c W[o,c]*pre[b,c,t] + bias[o]) for 2 batches.
PE matmul computes lhsT.T @ rhs with lhsT = W.T tiles [K=128, M=128] and
rhs = pre tiles [K=128, N=512]; accumulate 4 K-tiles into one PSUM bank,
then ScalarE applies out = 2*psum + 2*bias on eviction PSUM->SBUF.
"""

import os
import sys

for _p in ("/opt/trn_rl_repo", "/root/.axon_site/_ro/trn_rl_repo"):
    if os.path.isdir(_p) and _p not in sys.path:
        sys.path.append(_p)

from contextlib import ExitStack

import numpy as np

import concourse.bass as bass
import concourse.tile as tile
from concourse import bacc, mybir
from concourse.bass_utils import run_bass_kernel_spmd

B, C, T = 16, 512, 4096  # batch, channels (in == out), sequence
NCORES = 8
BPC = B // NCORES  # batches per core
P = 128
KT = C // P  # contraction tiles
MT = C // P  # output-channel tiles
NCHUNK = 512  # matmul moving-operand free dim (max for 4-byte dtypes)
NCH = T // NCHUNK
# Input DMA column chunks: small first chunks so the first matmul group's
# data lands early, bigger later ones to amortize DMA issue overhead.
XCS = [512, 512, 1024, 2048]
# Output store groups (in NCHUNK units) per batch: taper the last batch so the
# final DMAs after the last matmul are small.
OGS = {0: [4, 4], 1: [4, 2, 1, 1]}

# float32: exact, 4 cycles/row on PE. float32r (tf32): 1 cycle/row at N>=256.
MM_DTYPE = mybir.dt.float32r

LAST_RESULT = None  # BassKernelResults of the most recent run (for test harness)
_cache = {}


def _build(mm_dtype):
    # Bacc (not plain Bass): its finalize() runs move_matmul_waits_to_ldweights +
    # generate_event_semaphores, which walrus needs — an fp32 self-loading
    # matmul's implicit LDWEIGHTS tolerates only one semaphore wait.
    nc = bacc.Bacc("TRN2", target_bir_lowering=False, debug=False, num_devices=NCORES)
    # When running tf32 matmuls, the BIR verifier requires matmul inputs to be
    # produced as float32r; declaring the DRAM side as float32r (with the host
    # pre-rounding the payload to tf32) satisfies it without a device-side pass.
    in_dt = mm_dtype if mm_dtype == mybir.dt.float32r else mybir.dt.float32
    pre = nc.dram_tensor("pre", [BPC, C, T], in_dt, kind="ExternalInput").ap()
    wt = nc.dram_tensor("wt", [C, C], in_dt, kind="ExternalInput").ap()
    b2 = nc.dram_tensor("b2", [P, MT], mybir.dt.float32, kind="ExternalInput").ap()
    out = nc.dram_tensor("out", [BPC, C, T], mybir.dt.float32, kind="ExternalOutput").ap()

    with ExitStack() as ctx:
        tc = ctx.enter_context(tile.TileContext(nc))
        wpool = ctx.enter_context(tc.tile_pool(name="w", bufs=1))
        bpool = ctx.enter_context(tc.tile_pool(name="bias", bufs=1))
        xpool = ctx.enter_context(tc.tile_pool(name="x", bufs=2))
        opool = ctx.enter_context(tc.tile_pool(name="o", bufs=8))
        pspool = ctx.enter_context(tc.tile_pool(name="ps", bufs=8, space="PSUM"))

        # DMA issue order is consumption order: the first matmul group (batch 0,
        # nch 0, mt 0) needs x chunk 0 (4x256KB) + w column 0 (4x64KB) — those
        # eight transfers fill the eight HW queues' first round.
        xtiles_b0 = [[None] * KT for _ in range(len(XCS))]
        for kt in range(KT):
            x = xpool.tile([P, XCS[0]], in_dt, name=f"x_0_0_{kt}", tag=f"x0_{kt}")
            nc.sync.dma_start(x[:], pre[0, kt * P : (kt + 1) * P, 0 : XCS[0]])
            xtiles_b0[0][kt] = x

        # W.T resident in SBUF as 16 [128, 128] tiles; mt=0 column first.
        wtiles = [[None] * MT for _ in range(KT)]
        for mt in range(MT):
            for kt in range(KT):
                w = wpool.tile([P, P], in_dt, name=f"w_{kt}_{mt}", tag=f"w{kt}{mt}")
                nc.sync.dma_start(
                    w[:], wt[kt * P : (kt + 1) * P, mt * P : (mt + 1) * P]
                )
                wtiles[kt][mt] = w

        btile = bpool.tile([P, MT], mybir.dt.float32)
        nc.sync.dma_start(btile[:], b2[:])

        # nch -> (x tile index, column offset inside that tile)
        xmap = []
        off = 0
        for xi, xcols in enumerate(XCS):
            for o in range(0, xcols, NCHUNK):
                xmap.append((xi, o))
            off += xcols
        assert len(xmap) == NCH

        for b in range(BPC):
            xtiles = [[None] * KT for _ in range(len(XCS))]
            off = 0
            for xi, xcols in enumerate(XCS):
                if b == 0 and xi == 0:
                    xtiles[0] = xtiles_b0[0]
                    off += xcols
                    continue
                for kt in range(KT):
                    # Big trailing chunk single-buffered to stay inside SBUF;
                    # its reload for batch 1 overlaps batch 0's tail compute.
                    x = xpool.tile(
                        [P, xcols], in_dt, name=f"x_{b}_{xi}_{kt}",
                        tag=f"x{xi}_{kt}", bufs=(1 if xi == len(XCS) - 1 else 2),
                    )
                    nc.sync.dma_start(
                        x[:], pre[b, kt * P : (kt + 1) * P, bass.ds(off, xcols)]
                    )
                    xtiles[xi][kt] = x
                off += xcols

            nch = 0
            for og, osize in enumerate(OGS[b]):
                ocols = osize * NCHUNK
                otiles = [
                    opool.tile(
                        [P, ocols], mybir.dt.float32,
                        name=f"o_{b}_{og}_{mt}", tag="o",
                    )
                    for mt in range(MT)
                ]
                obase = nch * NCHUNK
                for j in range(osize):
                    xi, xoff = xmap[nch]
                    for mt in range(MT):
                        ps = pspool.tile([P, NCHUNK], mybir.dt.float32, tag="ps")
                        for kt in range(KT):
                            lhsT = wtiles[kt][mt][:]
                            rhs = xtiles[xi][kt][:, xoff : xoff + NCHUNK]
                            if mm_dtype != in_dt:
                                lhsT = lhsT.bitcast(mm_dtype)
                                rhs = rhs.bitcast(mm_dtype)
                            nc.tensor.matmul(
                                ps[:], lhsT, rhs, start=(kt == 0), stop=(kt == KT - 1)
                            )
                        # W is pre-scaled by 2 on the host, so only + 2*bias
                        # remains; alternate DVE/ACT so neither engine binds.
                        dst = otiles[mt][:, bass.ts(j, NCHUNK)]
                        bias_col = btile[:, mt : mt + 1]
                        if mt % 2 == 0:
                            nc.vector.tensor_scalar_add(dst, ps[:], bias_col)
                        else:
                            nc.scalar.activation(
                                dst,
                                ps[:],
                                mybir.ActivationFunctionType.Identity,
                                bias=bias_col,
                            )
                    nch += 1
                for mt in range(MT):
                    nc.gpsimd.dma_start(
                        out[b, mt * P : (mt + 1) * P, bass.ds(obase, ocols)],
                        otiles[mt][:],
                    )
    # The axon/PJRT exec path serializes nc as-is; finalize here so Bacc's
    # compile passes (register alloc, event-semaphore wait splitting) run.
    nc.finalize()
    return nc


def _round_tf32(a):
    """Round fp32 array to tf32 (10-bit mantissa), round-to-nearest-even."""
    u = a.view(np.uint32)
    r = u + (0xFFF + ((u >> 13) & 1))
    r &= np.uint32(0xFFFFE000)
    # NaN/Inf payloads must not be touched by the carry into the exponent
    special = (u & np.uint32(0x7F800000)) == np.uint32(0x7F800000)
    r[special] = u[special] & np.uint32(0xFFFFE000)
    return r.view(np.float32)


def kernel(pre, W_pre, b_pre):
    global LAST_RESULT
    pre = np.ascontiguousarray(pre, dtype=np.float32)
    # Fold the reference's final y+y into the weights/bias: out = (2W)x + 2b.
    wT = np.ascontiguousarray(np.asarray(W_pre, dtype=np.float32).T * 2.0)
    if MM_DTYPE == mybir.dt.float32r:
        pre = _round_tf32(pre)
        wT = _round_tf32(wT)
    b2 = np.ascontiguousarray(
        (2.0 * np.asarray(b_pre, dtype=np.float32)).reshape(MT, P).T
    )
    key = str(MM_DTYPE)
    if key not in _cache:
        _cache[key] = _build(MM_DTYPE)
    nc = _cache[key]
    in_maps = [
        {"pre": pre[i * BPC : (i + 1) * BPC], "wt": wT, "b2": b2}
        for i in range(NCORES)
    ]
    res = run_bass_kernel_spmd(nc, in_maps, list(range(NCORES)))
    LAST_RESULT = res
    return np.ascontiguousarray(
        np.concatenate([res.results[i]["out"] for i in range(NCORES)], axis=0),
        dtype=np.float32,
    )



# revision 2
# speedup vs baseline: 1.3785x; 1.3785x over previous
"""Trainium2 Bass kernel for y = 2*(einsum('bct,oc->bot', pre, W_pre) + b_pre).

Shapes (hardcoded): pre [16, 512, 4096] f32, W_pre [512, 512] f32, b_pre [512] f32.
Sharding: data-parallel over B across 8 cores (2 batches per core).

The kernel is DMA-bound at fp32 (33.6MB/core I/O vs ~330GB/s/core), so all
device I/O is bf16: pre and W are rounded to bf16 on the host, the output is
computed in fp32 PSUM, evicted to bf16 SBUF tiles, DMA'd out as bf16, and
upcast to fp32 on the host. Measured max-rel error ~3.8e-3 (gate 2e-2).

Per core: out[b, o, t] = 2*(sum_c W[o,c]*pre[b,c,t] + bias[o]) for 2 batches.
PE matmul computes lhsT.T @ rhs with lhsT = (2W).T bf16 tiles [K=128, M=128]
and rhs = pre bf16 tiles [K=128, N=512]; accumulate 4 K-tiles into one PSUM
bank, then DVE/ScalarE apply out = psum + 2*bias on eviction PSUM->SBUF(bf16).
"""

import os
import sys

for _p in ("/opt/trn_rl_repo", "/root/.axon_site/_ro/trn_rl_repo"):
    if os.path.isdir(_p) and _p not in sys.path:
        sys.path.append(_p)

from contextlib import ExitStack

import ml_dtypes
import numpy as np

import concourse.bass as bass
import concourse.tile as tile
from concourse import bacc, mybir
from concourse.bass_utils import run_bass_kernel_spmd

B, C, T = 16, 512, 4096  # batch, channels (in == out), sequence
NCORES = 8
BPC = B // NCORES  # batches per core
P = 128
KT = C // P  # contraction tiles
MT = C // P  # output-channel tiles
NCHUNK = 512  # matmul moving-operand free dim (max)
NCH = T // NCHUNK
# Input DMA column chunks: small first chunks so the first matmul group's
# data lands early, bigger later ones to amortize DMA issue overhead.
XCS = [512, 512, 1024, 2048]
# Output store groups (in NCHUNK units) per batch: taper the last batch so the
# final DMAs after the last matmul are small.
OGS = {0: [4, 4], 1: [4, 2, 1, 1]}

IN_DT = mybir.dt.bfloat16

LAST_RESULT = None  # BassKernelResults of the most recent run (for test harness)
_cache = {}


def _build():
    # Bacc (not plain Bass): its finalize() runs move_matmul_waits_to_ldweights +
    # generate_event_semaphores, which walrus needs.
    nc = bacc.Bacc("TRN2", target_bir_lowering=False, debug=False, num_devices=NCORES)
    pre = nc.dram_tensor("pre", [BPC, C, T], IN_DT, kind="ExternalInput").ap()
    wt = nc.dram_tensor("wt", [C, C], IN_DT, kind="ExternalInput").ap()
    b2 = nc.dram_tensor("b2", [P, MT], mybir.dt.float32, kind="ExternalInput").ap()
    out = nc.dram_tensor("out", [BPC, C, T], IN_DT, kind="ExternalOutput").ap()

    with ExitStack() as ctx:
        tc = ctx.enter_context(tile.TileContext(nc))
        wpool = ctx.enter_context(tc.tile_pool(name="w", bufs=1))
        bpool = ctx.enter_context(tc.tile_pool(name="bias", bufs=1))
        xpool = ctx.enter_context(tc.tile_pool(name="x", bufs=2))
        opool = ctx.enter_context(tc.tile_pool(name="o", bufs=8))
        pspool = ctx.enter_context(tc.tile_pool(name="ps", bufs=8, space="PSUM"))

        # DMA issue order is consumption order: the first matmul group (batch 0,
        # nch 0, mt 0) needs x chunk 0 + w column 0 first.
        xtiles_b0 = [[None] * KT for _ in range(len(XCS))]
        for kt in range(KT):
            x = xpool.tile([P, XCS[0]], IN_DT, name=f"x_0_0_{kt}", tag=f"x0_{kt}")
            nc.sync.dma_start(x[:], pre[0, kt * P : (kt + 1) * P, 0 : XCS[0]])
            xtiles_b0[0][kt] = x

        # (2W).T resident in SBUF as 16 [128, 128] tiles; mt=0 column first.
        wtiles = [[None] * MT for _ in range(KT)]
        for mt in range(MT):
            for kt in range(KT):
                w = wpool.tile([P, P], IN_DT, name=f"w_{kt}_{mt}", tag=f"w{kt}{mt}")
                nc.sync.dma_start(
                    w[:], wt[kt * P : (kt + 1) * P, mt * P : (mt + 1) * P]
                )
                wtiles[kt][mt] = w

        btile = bpool.tile([P, MT], mybir.dt.float32)
        nc.sync.dma_start(btile[:], b2[:])

        # nch -> (x tile index, column offset inside that tile)
        xmap = []
        off = 0
        for xi, xcols in enumerate(XCS):
            for o in range(0, xcols, NCHUNK):
                xmap.append((xi, o))
            off += xcols
        assert len(xmap) == NCH

        for b in range(BPC):
            xtiles = [[None] * KT for _ in range(len(XCS))]
            off = 0
            for xi, xcols in enumerate(XCS):
                if b == 0 and xi == 0:
                    xtiles[0] = xtiles_b0[0]
                    off += xcols
                    continue
                for kt in range(KT):
                    # Big trailing chunk single-buffered to stay inside SBUF;
                    # its reload for batch 1 overlaps batch 0's tail compute.
                    x = xpool.tile(
                        [P, xcols], IN_DT, name=f"x_{b}_{xi}_{kt}",
                        tag=f"x{xi}_{kt}", bufs=(1 if xi == len(XCS) - 1 else 2),
                    )
                    nc.sync.dma_start(
                        x[:], pre[b, kt * P : (kt + 1) * P, bass.ds(off, xcols)]
                    )
                    xtiles[xi][kt] = x
                off += xcols

            nch = 0
            for og, osize in enumerate(OGS[b]):
                ocols = osize * NCHUNK
                otiles = [
                    opool.tile(
                        [P, ocols], IN_DT,
                        name=f"o_{b}_{og}_{mt}", tag="o",
                    )
                    for mt in range(MT)
                ]
                obase = nch * NCHUNK
                for j in range(osize):
                    xi, xoff = xmap[nch]
                    for mt in range(MT):
                        ps = pspool.tile([P, NCHUNK], mybir.dt.float32, tag="ps")
                        for kt in range(KT):
                            nc.tensor.matmul(
                                ps[:],
                                wtiles[kt][mt][:],
                                xtiles[xi][kt][:, xoff : xoff + NCHUNK],
                                start=(kt == 0),
                                stop=(kt == KT - 1),
                            )
                        # W is pre-scaled by 2 on the host, so only + 2*bias
                        # remains; alternate DVE/ACT so neither engine binds.
                        dst = otiles[mt][:, bass.ts(j, NCHUNK)]
                        bias_col = btile[:, mt : mt + 1]
                        if mt % 2 == 0:
                            nc.vector.tensor_scalar_add(dst, ps[:], bias_col)
                        else:
                            nc.scalar.activation(
                                dst,
                                ps[:],
                                mybir.ActivationFunctionType.Identity,
                                bias=bias_col,
                            )
                    nch += 1
                for mt in range(MT):
                    nc.gpsimd.dma_start(
                        out[b, mt * P : (mt + 1) * P, bass.ds(obase, ocols)],
                        otiles[mt][:],
                    )
    # The axon/PJRT exec path serializes nc as-is; finalize here so Bacc's
    # compile passes (register alloc, event-semaphore wait splitting) run.
    nc.finalize()
    return nc


def kernel(pre, W_pre, b_pre):
    global LAST_RESULT
    bf16 = ml_dtypes.bfloat16
    pre_bf = np.ascontiguousarray(np.asarray(pre, dtype=np.float32)).astype(bf16)
    # Fold the reference's final y+y into the weights/bias: out = (2W)x + 2b.
    wT = np.ascontiguousarray(
        np.asarray(W_pre, dtype=np.float32).T * 2.0
    ).astype(bf16)
    b2 = np.ascontiguousarray(
        (2.0 * np.asarray(b_pre, dtype=np.float32)).reshape(MT, P).T
    )
    if "nc" not in _cache:
        _cache["nc"] = _build()
    nc = _cache["nc"]
    in_maps = [
        {"pre": pre_bf[i * BPC : (i + 1) * BPC], "wt": wT, "b2": b2}
        for i in range(NCORES)
    ]
    res = run_bass_kernel_spmd(nc, in_maps, list(range(NCORES)))
    LAST_RESULT = res
    return np.ascontiguousarray(
        np.concatenate([res.results[i]["out"] for i in range(NCORES)], axis=0)
    ).astype(np.float32)


# revision 5
# speedup vs baseline: 1.5616x; 1.1329x over previous
"""Trainium2 Bass kernel for y = 2*(einsum('bct,oc->bot', pre, W_pre) + b_pre).

Shapes (hardcoded): pre [16, 512, 4096] f32, W_pre [512, 512] f32, b_pre [512] f32.
Sharding: data-parallel over B across 8 cores (2 batches per core).

DMA-bound at fp32 (33.6MB/core I/O vs ~330GB/s/core), so all device I/O is
bf16: pre/W rounded to bf16 on the host, fp32 PSUM accumulation, eviction to
bf16 SBUF, bf16 output upcast to fp32 on the host. Max-rel err ~3.8e-3.

Schedule notes (from trace analysis):
- Each dma_start costs its sequencer ~600ns of DGE config, and one DMA's
  descriptors spread across all 16 HW engines — so transfers are packed:
  one DMA per (batch, column segment) carrying all 4 K-tiles ([P, KT, cols]
  SBUF layout), one DMA for all of W, one per output tile carrying all 4
  M-tiles.
- Column segments are small-first for batch 0 (PE starts sooner) and
  reversed (small-last) for batch 1 (shorter drain tail).
- The PE p-state ramps with activity; N_WARM tiny fp32 matmuls on a
  memset scratch tile burn the slow part of the ramp while the first
  real DMAs are still in flight.
- A dummy activation right after the bias DMA pre-triggers the Scalar
  engine's ACT_TABLE_LOAD (~1.3us) off the critical path.
"""

import os
import sys

for _p in ("/opt/trn_rl_repo", "/root/.axon_site/_ro/trn_rl_repo"):
    if os.path.isdir(_p) and _p not in sys.path:
        sys.path.append(_p)

from contextlib import ExitStack

import ml_dtypes
import numpy as np

import concourse.bass as bass
import concourse.tile as tile
from concourse import bacc, mybir
from concourse.bass_utils import run_bass_kernel_spmd

B, C, T = 16, 512, 4096  # batch, channels (in == out), sequence
NCORES = 8
BPC = B // NCORES  # batches per core
P = 128
KT = C // P  # contraction tiles
MT = C // P  # output-channel tiles
NCHUNK = 512  # max matmul moving-operand free dim

# Column segments (one packed DMA each). Batch 0 consumes them in this
# order (small first => first matmul starts early); batch 1 reversed
# (small last => short tail).
SEGS = [128, 384, 512, 1024, 2048]
assert sum(SEGS) == T

N_WARM = 24  # tiny warmup matmuls to ride the PE p-state ramp

IN_DT = mybir.dt.bfloat16

LAST_RESULT = None  # BassKernelResults of the most recent run (for test harness)
_cache = {}


def _chunks(cols):
    """Split a segment into matmul groups of <= NCHUNK columns."""
    out = []
    off = 0
    while off < cols:
        n = min(NCHUNK, cols - off)
        out.append((off, n))
        off += n
    return out


def _build():
    nc = bacc.Bacc("TRN2", target_bir_lowering=False, debug=False, num_devices=NCORES)
    pre = nc.dram_tensor("pre", [BPC, C, T], IN_DT, kind="ExternalInput").ap()
    wt = nc.dram_tensor("wt", [C, C], IN_DT, kind="ExternalInput").ap()
    b2 = nc.dram_tensor("b2", [P, MT], mybir.dt.float32, kind="ExternalInput").ap()
    out = nc.dram_tensor("out", [BPC, C, T], IN_DT, kind="ExternalOutput").ap()

    with ExitStack() as ctx:
        tc = ctx.enter_context(tile.TileContext(nc))
        wpool = ctx.enter_context(tc.tile_pool(name="w", bufs=1))
        bpool = ctx.enter_context(tc.tile_pool(name="bias", bufs=1))
        xpool = ctx.enter_context(tc.tile_pool(name="x", bufs=2))
        opool = ctx.enter_context(tc.tile_pool(name="o", bufs=3))
        pspool = ctx.enter_context(tc.tile_pool(name="ps", bufs=7, space="PSUM"))
        wmpool = ctx.enter_context(tc.tile_pool(name="wm", bufs=1, space="PSUM"))

        # --- PE warmup: ride the p-state ramp on garbage data -------------
        warm = bpool.tile([P, 4], mybir.dt.float32, name="warm_src")
        nc.vector.memset(warm[:], 0.0)
        wps = wmpool.tile([4, 4], mybir.dt.float32, name="warm_ps")
        for _ in range(N_WARM):
            nc.tensor.matmul(wps[:], warm[:], warm[:], start=True, stop=True)

        # --- weights + bias ----------------------------------------------
        # W.T packed as [P, KT, MT, P]: [p, kt, mt, m] = (2W)[mt*P+m, kt*P+p]
        wtile = wpool.tile([P, KT, MT, P], IN_DT, name="wt_all")
        nc.sync.dma_start(
            wtile[:], wt.rearrange("(kt p) (mt m) -> p kt mt m", kt=KT, mt=MT)
        )
        btile = bpool.tile([P, MT], mybir.dt.float32, name="bias")
        nc.scalar.dma_start(btile[:], b2[:])
        # Pre-trigger Scalar ACT_TABLE_LOAD off the critical path.
        scr = bpool.tile([P, 1], mybir.dt.float32, name="act_scr")
        nc.scalar.activation(
            scr[:], btile[:, 0:1], mybir.ActivationFunctionType.Identity
        )

        # --- x segment DMAs (issue = consumption order) -------------------
        seg_plan = {}  # b -> list of (base_col, cols, xtile)
        for b in range(BPC):
            segs = SEGS if b == 0 else SEGS[::-1]
            if b == 0:
                bases = [int(v) for v in np.cumsum([0] + list(segs))[:-1]]
            else:
                bases = [T - int(np.cumsum(segs)[i]) for i in range(len(segs))]
            plan = []
            for base, cols in zip(bases, segs):
                x = xpool.tile(
                    [P, KT, cols], IN_DT, name=f"x_{b}_{cols}_{base}",
                    tag=f"x{cols}", bufs=2,
                )
                nc.sync.dma_start(
                    x[:],
                    pre[b, :, bass.ds(base, cols)].rearrange(
                        "(kt p) t -> p kt t", kt=KT
                    ),
                )
                plan.append((int(base), cols, x))
            seg_plan[b] = plan

        # --- compute + eviction + output ---------------------------------
        # otile partitions: batch 0 -> two 2048-col tiles; batch 1 -> taper.
        evict_n = 0
        for b in range(BPC):
            plan = seg_plan[b]
            # flatten groups: (out_base_col, ncols, xtile, xoff)
            groups = []
            for base, cols, x in plan:
                for xoff, n in _chunks(cols):
                    groups.append((base + xoff, n, x, xoff))
            if b == 0:
                otile_groups = [groups[0:5], groups[5:9]]
            else:
                otile_groups = [
                    groups[0:4], groups[4:6], groups[6:7], groups[7:8], groups[8:9]
                ]
            for og in otile_groups:
                ocols = sum(g[1] for g in og)
                obase = min(g[0] for g in og)
                ot = opool.tile([P, MT, 2048], IN_DT, tag="o")
                for gbase, n, x, xoff in og:
                    for mt in range(MT):
                        ps = pspool.tile([P, NCHUNK], mybir.dt.float32, tag="ps")
                        for kt in range(KT):
                            nc.tensor.matmul(
                                ps[:, 0:n],
                                wtile[:, kt, mt, :],
                                x[:, kt, bass.ds(xoff, n)],
                                start=(kt == 0),
                                stop=(kt == KT - 1),
                            )
                        # W pre-scaled by 2 on host => only + 2*bias left;
                        # alternate DVE/ACT so neither engine binds.
                        dst = ot[:, mt, bass.ds(gbase - obase, n)]
                        bias_col = btile[:, mt : mt + 1]
                        if evict_n % 2 == 0:
                            nc.vector.tensor_scalar_add(dst, ps[:, 0:n], bias_col)
                        else:
                            nc.scalar.activation(
                                dst,
                                ps[:, 0:n],
                                mybir.ActivationFunctionType.Identity,
                                bias=bias_col,
                            )
                        evict_n += 1
                nc.gpsimd.dma_start(
                    out[b, :, bass.ds(obase, ocols)].rearrange(
                        "(mt p) t -> p mt t", mt=MT
                    ),
                    ot[:, :, 0:ocols],
                )
    nc.finalize()
    return nc


def kernel(pre, W_pre, b_pre):
    global LAST_RESULT
    bf16 = ml_dtypes.bfloat16
    pre_bf = np.ascontiguousarray(np.asarray(pre, dtype=np.float32)).astype(bf16)
    # Fold the reference's final y+y into the weights/bias: out = (2W)x + 2b.
    wT = np.ascontiguousarray(
        np.asarray(W_pre, dtype=np.float32).T * 2.0
    ).astype(bf16)
    b2 = np.ascontiguousarray(
        (2.0 * np.asarray(b_pre, dtype=np.float32)).reshape(MT, P).T
    )
    if "nc" not in _cache:
        _cache["nc"] = _build()
    nc = _cache["nc"]
    in_maps = [
        {"pre": pre_bf[i * BPC : (i + 1) * BPC], "wt": wT, "b2": b2}
        for i in range(NCORES)
    ]
    res = run_bass_kernel_spmd(nc, in_maps, list(range(NCORES)))
    LAST_RESULT = res
    return np.ascontiguousarray(
        np.concatenate([res.results[i]["out"] for i in range(NCORES)], axis=0)
    ).astype(np.float32)
